# revision 11
# baseline (speedup 1.0000x reference)
"""Causal self-attention + output projection + residual + LayerNorm on 8
Trainium2 NeuronCores.

Problem: B=4, S=2048, D=1024, H=16, dk=64 (fp32).

Sharding: core c = 2*b + g handles batch b with heads [8g, 8g+8) through the
attention AND the output projection (partial, own heads only); the pair
(2b, 2b+1) then ReduceScatters the partial projection outputs, so each core
finishes residual+LayerNorm on half the rows (core 2b: quarters 0 and 2 of
the sequence; core 2b+1: quarters 1 and 3). The host reassembles.

On-device layout notes:
 - Q^T / K^T are built directly in [head*dk, S] orientation (heads on the
   partition axis, 64 rows per head) so scores^T = K_tile^T . Q and the
   ctx matmul consume them without any transposes.
 - scores are computed transposed, [Sk partition, Sq free], so softmax's
   denominator comes out of the ctx matmul for free: V is stored with a
   ones-column appended per head ([Sk, 65]), so ctx_psum row 64 is the
   softmax denominator.
 - softmax skips the max-subtraction (scores are O(3), exp is safe in fp32,
   masked entries get -1e9 pre-exp and underflow to exactly 0).
 - causality: scores^T tiles are only computed on the valid triangle; the
   single diagonal 128x128 block per k-tile gets a -1e9 additive mask.
 - emission interleaves the QKV projection quarters with the attention
   chunks that only need earlier quarters, keeping PE and ACT overlapped.
 - LayerNorm runs on DVE/Pool (ACT only does the tiny sqrt), because ACT
   is saturated by the softmax exp.
"""

import numpy as np
from contextlib import ExitStack

import concourse.bass as bass
import concourse.mybir as mybir
import concourse.tile as tile
import bass_rust
from concourse.tile import ScopedClock
from concourse.bass_utils import run_bass_kernel_spmd

FP = mybir.dt.float32
BF = mybir.dt.bfloat16
AF = mybir.ActivationFunctionType

B, S, D, H, DK = 4, 2048, 1024, 16, 64
N_CORES = 8
HPC = H // 2          # heads per core = 8
NEG = -1e9
EPS = 1e-6

# ---------------------------------------------------------------------------
# Compat shims: this walrus build rejects instructions with more than one
# sync-wait condition; split extra waits onto same-engine NoOp carriers.
# ---------------------------------------------------------------------------
_ws_ctr = [0]


def _split_waits_in_ordered(ordered):
    for bb_name, insts in list(ordered.items()):
        new = []
        for inst in insts:
            si = inst.sync_info
            if si is None:
                new.append(inst)
                continue
            waits = list(si.on_wait)
            if len(waits) > 1:
                head = len(waits) - 1
                for i in range(head):
                    _ws_ctr[0] += 1
                    carrier = mybir.InstNoOp(
                        name=f"I-ws{_ws_ctr[0]}", engine=inst.engine
                    )
                    carrier.sync_info = bass_rust.SyncInfo(
                        on_wait=[waits[i]], on_update=[]
                    )
                    new.append(carrier)
                inst.sync_info = bass_rust.SyncInfo(
                    on_wait=waits[head:], on_update=si.on_update
                )
            new.append(inst)
        ordered[bb_name] = new


_orig_lower = tile.TileContext._lower_ordered_insts


def _patched_lower(self, ordered):
    _split_waits_in_ordered(ordered)
    return _orig_lower(self, ordered)


def _split_drain_and_barrier(self, tick_clock, wait_clock):
    drain_inst = self.nc.sync.drain()
    wait_clock.add_sem_waits(
        drain_inst.ins, ScopedClock({None: tick_clock.global_clock})
    )
    si = drain_inst.ins.sync_info
    waits = list(si.on_wait)
    if len(waits) > 1:
        drain_inst.ins.sync_info = bass_rust.SyncInfo(
            on_wait=waits[:1], on_update=si.on_update
        )
        for i in range(1, len(waits)):
            d2 = self.nc.sync.drain()
            d2.ins.sync_info = bass_rust.SyncInfo(
                on_wait=[waits[i]], on_update=[]
            )
    self.nc.all_engine_barrier()
    assert self.sems is not None
    popped = self.nc._tile_sem_poison_stack.pop()
    assert popped is self._sem_poison
    self.nc.clear_and_free_semaphores(list(self.sems.allocated().values()))
    self.nc.all_engine_barrier()


def _install_compat():
    tile.TileContext._lower_ordered_insts = _patched_lower
    tile.TileContext._drain_and_barrier = _split_drain_and_barrier


# ---------------------------------------------------------------------------
# Program builder
# ---------------------------------------------------------------------------
_cached_nc = {}


def _ap(tensor, offset, dims):
    return bass.AP(tensor=tensor, offset=offset, ap=[list(d) for d in dims])


def _mm(nc, out, lhsT, rhs, **kw):
    nc.tensor.matmul(out, lhsT, rhs, **kw)


def build_nc(reps=1):
    if reps in _cached_nc:
        return _cached_nc[reps]
    _install_compat()
    nc = bass.Bass("TRN2", target_bir_lowering=False, debug=False,
                   num_devices=N_CORES)

    xT = nc.dram_tensor("xT", [D, S], BF, kind="ExternalInput")
    xres = nc.dram_tensor("xres", [S // 2, D], FP, kind="ExternalInput")
    wq = nc.dram_tensor("wq", [D, 512], BF, kind="ExternalInput")
    wk = nc.dram_tensor("wk", [D, 512], BF, kind="ExternalInput")
    wv = nc.dram_tensor("wv", [D, 512], BF, kind="ExternalInput")
    bq = nc.dram_tensor("bq", [512], FP, kind="ExternalInput")
    bk = nc.dram_tensor("bk", [512], FP, kind="ExternalInput")
    bv = nc.dram_tensor("bv", [512], FP, kind="ExternalInput")
    wo = nc.dram_tensor("wo", [512, D], BF, kind="ExternalInput")
    gamma = nc.dram_tensor("gamma", [D], FP, kind="ExternalInput")
    beta = nc.dram_tensor("beta", [D], FP, kind="ExternalInput")
    mneg = nc.dram_tensor("mneg", [128, 128], FP, kind="ExternalInput")
    yout = nc.dram_tensor("y", [S // 2, D], FP, kind="ExternalOutput")

    NKT = S // 128            # 16 k-tiles over the sequence

    io = (xT, xres, wq, wk, wv, bq, bk, bv, wo, gamma, beta, mneg, yout)
    with tile.TileContext(nc) as tc:
        with ExitStack() as ctx:
            dram = ctx.enter_context(
                tc.tile_pool(name="dram", bufs=1, space="DRAM"))
            for r in range(reps):
                _emit_body(nc, tc, ctx, dram, io, r, NKT)

    _cached_nc[reps] = nc
    return nc


def _emit_body(nc, tc, ctx, dram, io, r, NKT):
    (xT, xres, wq, wk, wv, bq, bk, bv, wo, gamma, beta, mneg, yout) = io

    # ---- long-lived pools ----
    pqkv = tc.alloc_tile_pool(name=f"pqkv{r}", bufs=1)
    QT = pqkv.tile([128, 4, S], BF)        # [hd%128, pair, s]
    KT = pqkv.tile([128, 4, S], BF)
    V = pqkv.tile([128, NKT, HPC, 65], BF)  # per-head V + ones column
    ctxT = pqkv.tile([128, 4, S], BF)      # normalized ctx^T, packed pairs
    wo_t = pqkv.tile([128, 4, D], BF)      # own heads' Wo rows, by pair
    mneg_t = pqkv.tile([128, 128], FP)
    gam = pqkv.tile([128, D], FP)
    bet = pqkv.tile([128, D], FP)
    eps_t = pqkv.tile([128, 1], FP)
    nc.sync.dma_start(out=mneg_t, in_=mneg[:, :])
    nc.vector.memset(V[:, :, :, 64:65], 1.0)
    nc.sync.dma_start(out=gam, in_=_ap(gamma, 0, [[0, 128], [1, D]]))
    nc.sync.dma_start(out=bet, in_=_ap(beta, 0, [[0, 128], [1, D]]))
    nc.vector.memset(eps_t, EPS)
    nc.sync.dma_start(
        out=wo_t, in_=_ap(wo, 0, [[D, 128], [128 * D, 4], [1, D]]))

    p2e = tc.alloc_tile_pool(name=f"p2e{r}", bufs=4)      # expS tiles
    p2u = tc.alloc_tile_pool(name=f"p2u{r}", bufs=8)      # ctxU tiles
    p2n = tc.alloc_tile_pool(name=f"p2n{r}", bufs=2)      # bca tiles
    ppy = tc.alloc_tile_pool(name=f"ppy{r}", bufs=3)      # out-proj staging
    pln = tc.alloc_tile_pool(name=f"pln{r}", bufs=2)      # LN working tiles
    pmm = tc.alloc_tile_pool(name=f"pmm{r}", bufs=1, space="PSUM")

    # ---- phase-1 pools ----
    p1w = tc.alloc_tile_pool(name=f"p1w{r}", bufs=1)
    p1x = tc.alloc_tile_pool(name=f"p1x{r}", bufs=2)
    wq_t = p1w.tile([128, 8, 512], BF)
    wk_t = p1w.tile([128, 8, 512], BF)
    wv_t = p1w.tile([128, 8, 512], BF)
    for eng, wt, wd in ((nc.sync, wq_t, wq), (nc.gpsimd, wk_t, wk),
                        (nc.sync, wv_t, wv)):
        eng.dma_start(
            out=wt, in_=_ap(wd, 0, [[512, 128], [512 * 128, 8], [1, 512]]))
    bq_t = p1w.tile([128, 4], FP)
    bk_t = p1w.tile([128, 4], FP)
    nc.sync.dma_start(out=bq_t, in_=_ap(bq, 0, [[1, 128], [128, 4]]))
    nc.sync.dma_start(out=bk_t, in_=_ap(bk, 0, [[1, 128], [128, 4]]))
    bv_bc = p1w.tile([128, 8, 64], FP)
    nc.sync.dma_start(out=bv_bc, in_=_ap(bv, 0, [[0, 128], [64, 8], [1, 64]]))

    def emit_quarter(sq):
        # two half-tiles of x^T (D-tiles 0-3 / 4-7) for cheaper double-buffer
        xqh = []
        for dh in range(2):
            xq = p1x.tile([128, 4, 512], BF, name=f"xq{dh}", tag="xq", bufs=3)
            nc.gpsimd.dma_start(
                out=xq, in_=_ap(xT, 512 * sq + dh * 4 * 128 * S,
                                [[S, 128], [128 * S, 4], [1, 512]]))
            xqh.append(xq)
        for wt, bt, dst in ((wq_t, bq_t, QT), (wk_t, bk_t, KT)):
            for mt in range(4):
                ps = pmm.tile([128, 2, 512], FP, name="ps1", tag="mm", bufs=2)
                for d in range(8):
                    _mm(nc, ps[:, 0, :], wt[:, d, 128 * mt:128 * (mt + 1)],
                        xqh[d // 4][:, d % 4, :],
                        start=(d == 0), stop=(d == 7))
                nc.vector.tensor_scalar_add(
                    dst[:, mt, 512 * sq:512 * (sq + 1)], ps[:, 0, :],
                    bt[:, mt:mt + 1])
        for st in range(4):
            ps = pmm.tile([128, 2, 512], FP, name="ps1v", tag="mm", bufs=2)
            for d in range(8):
                _mm(nc, ps[:, 0, :],
                    xqh[d // 4][:, d % 4, 128 * st:128 * (st + 1)],
                    wv_t[:, d, :], start=(d == 0), stop=(d == 7))
            sg = 4 * sq + st
            nc.vector.tensor_add(
                V[:, sg, :, 0:64],
                ps[:, 0, :].rearrange("p (h e) -> p h e", h=HPC), bv_bc)

    def emit_chunk(p, w, jl, ctxU):
        """scores+exp+ctx for (pair p, wave w, half jl); accumulates the
        unnormalized ctx rows + denominator into ctxU[h][:, 512*jl:...]."""
        qlo = 1024 * w
        jlo = qlo + 512 * jl
        klast = (jlo + 512) // 128 - 1
        cps = [pmm.tile([65, 512], FP, name=f"cps{h}", tag="cps",
                        bufs=2) for h in range(2)]
        for k in range(klast + 1):
            clo = max(qlo, 128 * k)
            cstart = max(clo, jlo)
            blen = jlo + 512 - cstart
            doff = cstart - jlo
            s2 = pmm.tile([128, 2, 512], FP, name="s2", tag="mm", bufs=2)
            for h in range(2):
                rows = slice(64 * h, 64 * h + 64)
                _mm(nc, s2[:, h, 0:blen],
                    KT[rows, p, 128 * k:128 * (k + 1)],
                    QT[rows, p, cstart:cstart + blen],
                    start=True, stop=True, tile_position=(64 * h, 0))
            if cstart == 128 * k:
                dv = s2[:, :, 0:128]
                mb = _ap(mneg_t.tensor, mneg_t.offset,
                         [mneg_t.ap[0], [0, 2], mneg_t.ap[1]])
                nc.vector.tensor_add(dv, dv, mb)
            expS = p2e.tile([128, 2, 512], BF, name="expS", tag="expS")
            nc.scalar.activation(expS[:, :, 0:blen], s2[:, :, 0:blen],
                                 AF.Exp)
            for h in range(2):
                HH = 2 * p + h
                _mm(nc, cps[h][:, doff:doff + blen], V[:, k, HH, :],
                    expS[:, h, 0:blen],
                    start=(k == 0), stop=(k == klast))
        for h in range(2):
            nc.vector.tensor_copy(
                ctxU[h][:, 512 * jl:512 * (jl + 1)], cps[h])

    def emit_normalize(p, w, ctxU):
        """divide ctx rows by the denominator row and pack into ctxT."""
        qlo = 1024 * w
        HS = 1024
        dend = dram.tile([2, HS], BF, name=f"dend{r}_{p}_{w}")
        nc.sync.dma_start(out=dend[0:1, :], in_=ctxU[0][64:65, :])
        nc.gpsimd.dma_start(out=dend[1:2, :], in_=ctxU[1][64:65, :])
        bca = p2n.tile([64, 2, HS], BF, name="bca", tag="bca", bufs=2)
        for h, eng in ((0, nc.sync), (1, nc.gpsimd)):
            eng.dma_start(
                out=bca[:, h, :],
                in_=_ap(dend.tensor, dend.offset + h * HS, [[0, 64], [1, HS]]))
        with nc.allow_low_precision(reason="bf16 softmax denominators"):
            nc.vector.reciprocal(bca, bca)
        nc.vector.tensor_mul(
            ctxT[0:64, p, qlo:qlo + HS], ctxU[0][0:64, :], bca[:, 0, :])
        nc.gpsimd.tensor_mul(
            ctxT[64:128, p, qlo:qlo + HS], ctxU[1][0:64, :], bca[:, 1, :])

    py_dram = [dram.tile([1024, D], BF, name=f"py{r}_{w}") for w in range(2)]
    pyr_dram = [dram.tile([512, D], BF, name=f"pyr{r}_{w}") for w in range(2)]

    def emit_outproj(w, st):
        """partial out-proj for Sq tile st of wave w, own 8 heads only."""
        gst = 8 * w + st
        pyt = ppy.tile([128, D], BF, name="pyt", tag="pyt", bufs=3)
        for dsl in range(2):
            ps = pmm.tile([128, 512], FP, name="ps3", tag="ps3", bufs=2)
            for p in range(4):
                _mm(nc, ps, ctxT[:, p, 128 * gst:128 * (gst + 1)],
                    wo_t[:, p, 512 * dsl:512 * (dsl + 1)],
                    start=(p == 0), stop=(p == 3))
            nc.vector.tensor_copy(pyt[:, 512 * dsl:512 * (dsl + 1)], ps)
        nc.sync.dma_start(
            out=py_dram[w][128 * st:128 * (st + 1), :], in_=pyt)

    def emit_rs(w):
        nc.gpsimd.collective_compute(
            "ReduceScatter", mybir.AluOpType.add,
            replica_groups=[[0, 1], [2, 3], [4, 5], [6, 7]],
            ins=[py_dram[w].opt()], outs=[pyr_dram[w].opt()])

    def emit_ln(w, i):
        """residual + LayerNorm for own quarter tile i of wave w."""
        row0 = 512 * w + 128 * i
        pyt = pln.tile([128, D], BF, name="lnp", tag="lnp", bufs=2)
        nc.sync.dma_start(out=pyt, in_=pyr_dram[w][128 * i:128 * (i + 1), :])
        xr = pln.tile([128, D], FP, name="xr", tag="xr", bufs=2)
        nc.gpsimd.dma_start(out=xr, in_=xres[row0:row0 + 128, :])
        yt = pln.tile([128, D], FP, name="yt", tag="yt", bufs=2)
        nc.vector.tensor_add(yt, xr, pyt)
        stats = pln.tile([128, 2, 6], FP, name="stats", tag="stats")
        for hhalf in range(2):
            nc.vector.bn_stats(
                stats[:, hhalf, :], yt[:, 512 * hhalf:512 * (hhalf + 1)])
        mv = pln.tile([128, 2], FP, name="mv", tag="mv")
        nc.vector.bn_aggr(mv, stats)
        nmu = pln.tile([128, 1], FP, name="nmu", tag="nmu")
        nc.vector.tensor_scalar_mul(nmu, mv[:, 0:1], -1.0)
        sd = pln.tile([128, 1], FP, name="sd", tag="sd")
        nc.scalar.activation(sd, mv[:, 1:2], AF.Sqrt, bias=eps_t, scale=1.0)
        rstd = pln.tile([128, 1], FP, name="rstd", tag="rstd")
        nc.vector.reciprocal(rstd, sd)
        nc.vector.tensor_scalar_add(yt, yt, nmu)
        nc.gpsimd.tensor_scalar_mul(yt, yt, rstd)
        nc.gpsimd.tensor_mul(yt, yt, gam)
        ot = pln.tile([128, D], FP, name="ot", tag="ot")
        nc.vector.tensor_add(ot, yt, bet)
        nc.gpsimd.dma_start(out=yout[row0:row0 + 128, :], in_=ot)

    # ---------------- emission schedule ----------------
    emit_quarter(0)
    emit_quarter(1)
    cu = {}
    for p in range(4):
        cu[p] = [p2u.tile([65, 1024], BF, name=f"cu0_{p}{h}", tag="cu",
                          bufs=8) for h in range(2)]
        emit_chunk(p, 0, 0, cu[p])
        emit_chunk(p, 0, 1, cu[p])
    emit_quarter(2)
    for p in range(4):
        emit_normalize(p, 0, cu[p])
    for st in range(8):
        emit_outproj(0, st)
    emit_rs(0)
    cu1 = {}
    for p in range(4):
        cu1[p] = [p2u.tile([65, 1024], BF, name=f"cu1_{p}{h}", tag="cu",
                           bufs=8) for h in range(2)]
        emit_chunk(p, 1, 0, cu1[p])
    emit_quarter(3)
    for p in range(4):
        emit_chunk(p, 1, 1, cu1[p])
    for i in range(4):
        emit_ln(0, i)
    for p in range(4):
        emit_normalize(p, 1, cu1[p])
    for st in range(8):
        emit_outproj(1, st)
    emit_rs(1)
    for i in range(4):
        emit_ln(1, i)

    p1x.release()
    p1w.release()
    pmm.release()
    pln.release()
    ppy.release()
    p2n.release()
    p2u.release()
    p2e.release()
    pqkv.release()


# ---------------------------------------------------------------------------
# Host-side entry point
# ---------------------------------------------------------------------------
def make_in_maps(x, Wq, bq, Wk, bk, Wv, bv, Wo, bo, gamma, beta):
    import ml_dtypes
    bf16 = ml_dtypes.bfloat16
    x = np.asarray(x, np.float32)
    WqS = (np.asarray(Wq, np.float32) / np.sqrt(np.float32(DK))).reshape(D, H * DK)
    bqS = (np.asarray(bq, np.float32) / np.sqrt(np.float32(DK))).reshape(H * DK)
    WkF = np.asarray(Wk, np.float32).reshape(D, H * DK)
    bkF = np.asarray(bk, np.float32).reshape(H * DK)
    WvF = np.asarray(Wv, np.float32).reshape(D, H * DK)
    bvF = np.asarray(bv, np.float32).reshape(H * DK)
    WoF = np.ascontiguousarray(
        np.asarray(Wo, np.float32).reshape(H * DK, D).astype(bf16))
    boF = np.asarray(bo, np.float32)
    gF = np.ascontiguousarray(np.asarray(gamma, np.float32))
    btF = np.ascontiguousarray(np.asarray(beta, np.float32))
    kk = np.arange(128)[:, None]
    qq = np.arange(128)[None, :]
    mneg = np.where(kk <= qq, 0.0, NEG).astype(np.float32)

    in_maps = []
    for c in range(N_CORES):
        b, g = divmod(c, 2)
        cols = slice(512 * g, 512 * (g + 1))
        xb = x[b] + boF[None, :]
        xres_q = np.ascontiguousarray(
            np.concatenate(
                [xb[512 * g:512 * (g + 1)],
                 xb[1024 + 512 * g:1024 + 512 * (g + 1)]], axis=0))
        in_maps.append({
            "xT": np.ascontiguousarray(x[b].T.astype(bf16)),
            "xres": xres_q,
            "wq": np.ascontiguousarray(WqS[:, cols].astype(bf16)),
            "wk": np.ascontiguousarray(WkF[:, cols].astype(bf16)),
            "wv": np.ascontiguousarray(WvF[:, cols].astype(bf16)),
            "bq": np.ascontiguousarray(bqS[cols]),
            "bk": np.ascontiguousarray(bkF[cols]),
            "bv": np.ascontiguousarray(bvF[cols]),
            "wo": np.ascontiguousarray(WoF[512 * g:512 * (g + 1)]),
            "gamma": gF,
            "beta": btF,
            "mneg": mneg,
        })
    return in_maps


def kernel(x, Wq, bq, Wk, bk, Wv, bv, Wo, bo, gamma, beta):
    nc = build_nc()
    in_maps = make_in_maps(x, Wq, bq, Wk, bk, Wv, bv, Wo, bo, gamma, beta)
    r = run_bass_kernel_spmd(nc, in_maps, list(range(N_CORES)))
    out = np.empty((B, S, D), np.float32)
    for b in range(B):
        y0 = r.results[2 * b]["y"]
        y1 = r.results[2 * b + 1]["y"]
        out[b, 0:512] = y0[0:512]
        out[b, 512:1024] = y1[0:512]
        out[b, 1024:1536] = y0[512:1024]
        out[b, 1536:2048] = y1[512:1024]
    return out
